# revision 41
# baseline (speedup 1.0000x reference)
"""Bass/Trainium2 kernel for BasicGNNLayer (COO SpMM + mean aggregation + residual).

    out = features + (segment_sum(features[col], row) / clip(deg, 1)) .

Strategy (8 NeuronCores, SPMD, no collectives):
  - Destination-shard nodes: core m owns a 12544-row slab (98 tiles of 128).
  - Host sorts edges by dst row into 128-node tiles; within a tile edges are
    bucketed into 4 OVERLAPPING source windows of 32768 rows (int16 index
    reach). A per-tile balancer assigns window-flexible edges to even out the
    4 buckets, halving cap overflow. Each (tile, window) bucket is capped at
    CAP chunks of 128 edges; the excess spills into per-(7-tile-group, window)
    overflow chunks shared by the group's tiles.
  - The hard bottleneck is Q7 SWDGE descriptor generation (~7-9ns/index,
    data-independent). The 4 windows' gathers go to SWDGE queues 0-3, which
    execute on DIFFERENT Q7 core pairs concurrently (~3.4x).
  - Per (group, window): ONE dma_gather of (GRP*CAP+GOC)*128 indices (256B
    rows from the [N, 128] bf16 table). Per tile: the Scalar engine
    materializes the dst-offset row as a dense [P, KT, 128] tile (so the DVE
    is_equal runs in 2x_1P mode and stays off the Q7's shared SBUF port),
    then chained matmuls S.T @ G accumulate the segment sums in PSUM. The
    epilogue fuses host-precomputed 1/max(deg,1) scaling + residual add in
    one DVE op. Group g's compute hides under group g+1's gather wave;
    gpool bufs=3 keeps the gather waves back-to-back (299ns gaps).
"""

import os
import sys

for _p in ("/opt/trn_rl_repo", "/root/.axon_site/_ro/trn_rl_repo"):
    if os.path.isdir(_p) and _p not in sys.path:
        sys.path.insert(0, _p)

import numpy as np
import ml_dtypes

P = 128  # SBUF partitions
NSHARD = 4  # gather-table shards (int16 index limit)
CAP = 4  # capped chunks per (tile, shard) bucket


def _pick_grp(T):
    for g in range(8, 0, -1):
        if T % g == 0:
            return g
    return 1


def _shard_size(N):
    s = (N + NSHARD - 1) // NSHARD
    assert s <= 32768, "int16 index limit"
    return s


WW = 1 << 15  # shard window width (int16 index limit)


def _window_starts(N):
    # 4 overlapping windows covering [0, N); overlap gives the balancer
    # freedom to even out per-(tile, shard) bucket loads.
    if N > WW:
        base = N - WW
        return np.array([round(i * base / (NSHARD - 1)) for i in range(NSHARD)])
    q = (N + NSHARD - 1) // NSHARD
    return np.array([min(i * q, N - 1) for i in range(NSHARD)])


def _assign_shards(row, col, N):
    """Per-edge shard in 0..3, balancing each dst tile's 4 buckets."""
    starts = _window_starts(N)
    tile = row // P
    s_hi = np.searchsorted(starts, col, side="right") - 1
    flex = (s_hi > 0) & (col < starts[s_hi - 1] + WW)
    ntile = int(tile.max()) + 1
    fixed_cnt = np.zeros((ntile, NSHARD), np.int64)
    np.add.at(fixed_cnt, (tile[~flex], s_hi[~flex]), 1)
    flex_cnt = np.zeros((ntile, NSHARD - 1), np.int64)
    np.add.at(flex_cnt, (tile[flex], s_hi[flex] - 1), 1)

    tvals = np.zeros((ntile, NSHARD - 1), np.int64)
    for tt in range(ntile):
        f = fixed_cnt[tt]
        x = flex_cnt[tt]
        lo, hi = -(-(f.sum() + x.sum()) // NSHARD), int(f.sum() + x.sum())
        best = None
        while lo <= hi:
            M = (lo + hi) // 2
            t = np.zeros(NSHARD - 1, np.int64)
            carry = 0
            ok = True
            for p in range(NSHARD - 1):
                room = M - f[p] - carry
                if room < 0:
                    ok = False
                    break
                t[p] = min(x[p], room)
                carry = x[p] - t[p]
            if ok and f[NSHARD - 1] + carry <= M:
                best = t.copy()
                hi = M - 1
            else:
                lo = M + 1
        assert best is not None
        tvals[tt] = best

    # rank flex edges within (tile, pair) by col; first tvals go to lower shard
    shard = s_hi.copy()
    fi = np.where(flex)[0]
    pair = s_hi[fi] - 1
    key = tile[fi] * (NSHARD - 1) + pair
    order = np.lexsort((col[fi], key))
    ks = key[order]
    grp_start = np.zeros(ntile * (NSHARD - 1), np.int64)
    cnt = np.bincount(ks, minlength=ntile * (NSHARD - 1))
    grp_start[1:] = np.cumsum(cnt)[:-1]
    rank = np.arange(fi.shape[0]) - grp_start[ks]
    down = rank < tvals.reshape(-1)[ks]
    pair_o = pair[order]
    sh_f = np.where(down, pair_o, pair_o + 1)
    shard[fi[order]] = sh_f
    return shard, starts


# ---------------------------------------------------------------- host side


def preprocess(features, row, col, n_cores):
    """Build per-core input maps. Returns (in_maps, meta)."""
    N, D = features.shape
    E = row.shape[0]
    npc = ((N + n_cores - 1) // n_cores + P - 1) // P * P
    T = npc // P
    GRP = _pick_grp(T)
    NG = T // GRP
    SS = _shard_size(N)

    row = np.asarray(row).astype(np.int64)
    col = np.asarray(col).astype(np.int64)

    shard, wst = _assign_shards(row, col, N)
    gts0 = (row // P) * NSHARD + shard  # global (tile, shard) bucket id
    # secondary col sort inside each bucket -> ascending gather addresses
    # (better HBM row-buffer locality for the SDMA drain)
    order = np.argsort(gts0 * (1 << 17) + col, kind="stable")
    rs = row[order]
    cs = col[order]
    sh = shard[order]
    gts = gts0[order]

    n_gts = (n_cores * T) * NSHARD
    cnt = np.bincount(gts, minlength=n_gts)
    Cs = max(1, int((cnt.max() + P - 1) // P))
    cap = min(CAP, Cs)

    # rank of each edge within its bucket
    starts = np.zeros(n_gts, np.int64)
    starts[1:] = np.cumsum(cnt)[:-1]
    pos = np.arange(E) - starts[gts]

    # overflow edges (pos >= cap*128) pool per (core, group, shard)
    ovf = pos >= cap * P
    gtile = rs // P  # global tile id
    core_of = gtile // T
    g_of = (gtile % T) // GRP
    pool = (core_of * NG + g_of) * NSHARD + sh  # global pool id
    n_pools = n_cores * NG * NSHARD
    ovf_pool = pool[ovf]
    po = np.argsort(ovf_pool, kind="stable")
    pcnt = np.bincount(ovf_pool, minlength=n_pools)
    pstart = np.zeros(n_pools, np.int64)
    pstart[1:] = np.cumsum(pcnt)[:-1]
    ovrank_sub = np.empty(ovf_pool.shape[0], np.int64)
    ovrank_sub[po] = np.arange(ovf_pool.shape[0]) - pstart[ovf_pool[po]]
    ovrank = np.zeros(E, np.int64)
    ovrank[np.where(ovf)[0]] = ovrank_sub
    GOC = int((pcnt.max() + P - 1) // P) if ovf.any() else 0
    OC2 = 0
    ov2 = np.zeros(E, bool)
    ovrank2 = np.zeros(E, np.int64)
    if GOC > 1:
        GOC = 1
        ov2[np.where(ovf)[0]] = ovrank_sub >= P
        pool2 = core_of * NSHARD + sh
        p2 = pool2[ov2]
        po2 = np.argsort(p2, kind="stable")
        p2cnt = np.bincount(p2, minlength=n_cores * NSHARD)
        p2start = np.zeros(n_cores * NSHARD, np.int64)
        p2start[1:] = np.cumsum(p2cnt)[:-1]
        r2 = np.empty(p2.shape[0], np.int64)
        r2[po2] = np.arange(p2.shape[0]) - p2start[p2[po2]]
        ovrank2[np.where(ov2)[0]] = r2
        OC2 = int((p2cnt.max() + P - 1) // P) if ov2.any() else 0

    CPT = NSHARD * cap  # capped chunks per tile
    KT = CPT + NSHARD * GOC + NSHARD * OC2  # matmul chunks per tile
    CC = GRP * cap + GOC  # gather chunks per (group, shard) call
    CL = CC * P  # indices per call
    CLW = CL // 16

    tab = np.zeros((N, 2 * D), ml_dtypes.bfloat16)
    tab[:, :D] = features.astype(ml_dtypes.bfloat16)
    deg = np.bincount(row, minlength=N).astype(np.float32)
    recip_full = 1.0 / np.maximum(deg, 1.0)
    iota = np.ascontiguousarray(
        np.broadcast_to(np.arange(P, dtype=np.float32), (P, P))
    ).astype(ml_dtypes.bfloat16)

    bounds = np.searchsorted(gts, np.arange(n_cores + 1) * T * NSHARD)

    in_maps = []
    for m in range(n_cores):
        lo, hi = bounds[m], bounds[m + 1]
        local = rs[lo:hi] - m * npc
        tile = local // P  # tile within core
        shd = sh[lo:hi]
        pp_ = pos[lo:hi]
        ov_ = ovf[lo:hi]
        ovr = ovrank[lo:hi]  # only valid where ov_
        ov2_ = ov2[lo:hi]
        ovr2 = ovrank2[lo:hi]
        g = tile // GRP
        tin = tile % GRP

        # --- capped slots
        cm = ~ov_
        cc = pp_[cm] // P
        cp = pp_[cm] % P
        # rowv column layout per tile t: [s*cap + c | CPT + s*GOC + co]
        rowv = np.full((P, T * KT), -1.0, np.float32)
        rowv[cp, (tile[cm] * KT + shd[cm] * cap + cc)] = (
            local[cm] - tile[cm] * P
        ).astype(np.float32)
        # idx flat position within call (g, s): (tin*cap + cc)*128 + p
        idx_flat = np.zeros(NG * NSHARD * CL, np.int16)
        call = g * NSHARD + shd
        q = (call[cm] * CC + tin[cm] * cap + cc) * P + cp
        idx_flat[q] = (cs[lo:hi][cm] - wst[shd[cm]]).astype(np.int16)

        # --- group overflow slots (first level)
        lvl1 = ov_ & ~ov2_
        if GOC:
            co = ovr[lvl1] // P
            op_ = ovr[lvl1] % P
            rowv[op_, (tile[lvl1] * KT + CPT + shd[lvl1] * GOC + co)] = (
                local[lvl1] - tile[lvl1] * P
            ).astype(np.float32)
            qo = (call[lvl1] * CC + GRP * cap + co) * P + op_
            idx_flat[qo] = (cs[lo:hi][lvl1] - wst[shd[lvl1]]).astype(np.int16)

        ncalls = NG * NSHARD
        w = idx_flat.reshape(ncalls, CLW, 16)
        w = np.ascontiguousarray(np.transpose(w, (2, 0, 1))).reshape(16, ncalls * CLW)
        idx16 = np.ascontiguousarray(np.tile(w, (8, 1)))

        # --- core overflow (second level): 4 mini-calls appended after
        if OC2:
            idx_ov = np.zeros(NSHARD * OC2 * P, np.int16)
            co2 = ovr2[ov2_] // P
            op2 = ovr2[ov2_] % P
            rowv[op2, (tile[ov2_] * KT + CPT + NSHARD * GOC + shd[ov2_] * OC2 + co2)] = (
                local[ov2_] - tile[ov2_] * P
            ).astype(np.float32)
            q2 = (shd[ov2_] * OC2 + co2) * P + op2
            idx_ov[q2] = (cs[lo:hi][ov2_] - wst[shd[ov2_]]).astype(np.int16)
            OW2 = OC2 * P // 16
            w2 = idx_ov.reshape(NSHARD, OW2, 16)
            w2 = np.ascontiguousarray(np.transpose(w2, (2, 0, 1))).reshape(16, NSHARD * OW2)
            idx16 = np.concatenate([idx16, np.tile(w2, (8, 1))], axis=1)

        base = m * npc
        valid = max(0, min(npc, N - base))
        slab = np.zeros((T * P, D), np.float32)
        slab[:valid] = features[base : base + valid]
        feats_loc = np.ascontiguousarray(
            slab.reshape(T, P, D).transpose(1, 0, 2).reshape(P, T * D)
        )
        slabr = np.ones(T * P, np.float32)
        slabr[:valid] = recip_full[base : base + valid]
        recip_loc = np.ascontiguousarray(slabr.reshape(T, P).T)

        in_maps.append(
            {
                "tab": tab,
                "feats_loc": feats_loc,
                "idx16": idx16,
                "rowv": rowv.astype(ml_dtypes.bfloat16),
                "iota": iota,
                "recip": recip_loc,
            }
        )

    meta = dict(N=N, D=D, E=E, npc=npc, T=T, cap=cap, GOC=GOC, OC2=OC2, KT=KT,
                CC=CC, NG=NG, SS=SS, GRP=GRP, n_cores=n_cores,
                starts=[int(s) for s in wst])
    return in_maps, meta


def postprocess(results, meta):
    N, D, npc, T = meta["N"], meta["D"], meta["npc"], meta["T"]
    outs = []
    for m, res in enumerate(results):
        o = res["out"].reshape(P, T, D).transpose(1, 0, 2).reshape(T * P, D)
        valid = max(0, min(npc, N - m * npc))
        outs.append(o[:valid])
    return np.concatenate(outs, axis=0)


# -------------------------------------------------------------- device side


def build(meta):
    import concourse.bass as bass  # noqa: F401
    import concourse.bacc as bacc
    import concourse.mybir as mybir
    from concourse.tile import TileContext

    N, D, T = meta["N"], meta["D"], meta["T"]
    cap, GOC, KT, CC = meta["cap"], meta["GOC"], meta["KT"], meta["CC"]
    OC2 = meta["OC2"]
    NG, SS, GRP = meta["NG"], meta["SS"], meta["GRP"]
    starts = meta["starts"]
    W = 2 * D  # table row width (128)
    bf16 = mybir.dt.bfloat16
    f32 = mybir.dt.float32

    nc = bacc.Bacc(num_swdge_queues=4)
    tab = nc.dram_tensor("tab", [N, W], bf16, kind="ExternalInput")
    fl = nc.dram_tensor("feats_loc", [P, T * D], f32, kind="ExternalInput")
    NIX = NG * NSHARD * CC * P // 16 + NSHARD * OC2 * P // 16
    ix = nc.dram_tensor("idx16", [P, NIX], mybir.dt.int16, kind="ExternalInput")
    rv = nc.dram_tensor("rowv", [P, T * KT], bf16, kind="ExternalInput")
    io = nc.dram_tensor("iota", [P, P], bf16, kind="ExternalInput")
    rc = nc.dram_tensor("recip", [P, T], f32, kind="ExternalInput")
    ot = nc.dram_tensor("out", [P, T * D], f32, kind="ExternalOutput")

    CLW = CC * P // 16

    with TileContext(nc) as tc:
        with (
            tc.tile_pool(name="const", bufs=1) as cpool,
            tc.tile_pool(name="gat", bufs=3) as gpool,
            tc.tile_pool(name="rep", bufs=5) as rpool,
            tc.tile_pool(name="sel", bufs=5) as spool,
            tc.tile_pool(name="eplg", bufs=4) as epool,
            tc.tile_pool(name="acc", bufs=8, space="PSUM") as ppool,
        ):
            iota_sb = cpool.tile([P, P], bf16, tag="iota")
            nc.sync.dma_start(out=iota_sb[:, :], in_=io[:, :])
            # all group index tables resident up-front: no per-group idx DMA
            # on the critical chain; group 0's slice loads separately first so
            # the first gather wave is not gated on the full 3 MB transfer
            ixall = cpool.tile([P, NG * NSHARD * CLW], mybir.dt.int16, tag="ixall")
            nc.sync.dma_start(
                out=ixall[:, : NSHARD * CLW], in_=ix[:, : NSHARD * CLW]
            )
            if NG > 1:
                nc.sync.dma_start(
                    out=ixall[:, NSHARD * CLW : NG * NSHARD * CLW],
                    in_=ix[:, NSHARD * CLW : NG * NSHARD * CLW],
                )
            # dense [P, KT, P] iota replica: gives the is_equal two step-1
            # bf16 operands so the DVE can run in 2x_1P mode.
            iota_rep = cpool.tile([P, KT, P], bf16, tag="iota_rep")
            nc.scalar.activation(
                out=iota_rep[:, :, :],
                in_=iota_sb[:, :].unsqueeze(1).to_broadcast([P, KT, P]),
                func=mybir.ActivationFunctionType.Copy,
            )
            # bulk constants go through the Activation HWDGE ring so the Sync
            # ring can issue the first group's index loads immediately
            row_sb = cpool.tile([P, T * KT], bf16, tag="rowsb")
            nc.scalar.dma_start(out=row_sb[:, :], in_=rv[:, :])
            slab_sb = cpool.tile([P, T * D], f32, tag="slab")
            nc.scalar.dma_start(out=slab_sb[:, :], in_=fl[:, :])
            rc_sb = cpool.tile([P, T], f32, tag="recip")
            nc.scalar.dma_start(out=rc_sb[:, :], in_=rc[:, :])
            Gov = []
            if OC2:
                OW2 = OC2 * P // 16
                ixo = cpool.tile([P, NSHARD * OW2], mybir.dt.int16, tag="ixo")
                nc.sync.dma_start(
                    out=ixo[:, :],
                    in_=ix[:, NG * NSHARD * CC * P // 16 :],
                )
                for s in range(NSHARD):
                    Gv = cpool.tile([P, OC2, W], bf16, tag=f"Gov{s}")
                    nc.gpsimd.dma_gather(
                        out_ap=Gv[:, :, :],
                        in_ap=tab[starts[s] : min(N, starts[s] + WW), :],
                        idxs_ap=ixo[:, s * OW2 : (s + 1) * OW2],
                        num_idxs=OC2 * P,
                        num_idxs_reg=OC2 * P,
                        elem_size=W,
                        single_packet=False,
                        queue_num=s % 4,
                    )
                    Gov.append(Gv)

            for g in range(NG):
                Gs = []
                for s in range(NSHARD):
                    Gt = gpool.tile([P, CC, W], bf16, tag=f"G{s}")
                    call = g * NSHARD + s
                    nc.gpsimd.dma_gather(
                        out_ap=Gt[:, :, :],
                        in_ap=tab[starts[s] : min(N, starts[s] + WW), :],
                        idxs_ap=ixall[:, call * CLW : (call + 1) * CLW],
                        num_idxs=CC * P,
                        num_idxs_reg=CC * P,
                        elem_size=W,
                        single_packet=False,
                        queue_num=s % 4,
                    )
                    Gs.append(Gt)
                og = epool.tile([P, GRP * D], f32, tag="og")
                # Epilogues trail the S-build/matmul of their tile by 2 in the
                # DVE stream: eplg(t) then never blocks is_eq(t+1) on tile t's
                # matmul chain (the serial DVE->PE->DVE cycle that otherwise
                # paces the group at ~3us/tile), while keeping the DVE work
                # spread out instead of bursted (bursts slow the Q7 desc-gen).
                DLY = 4
                psums = []
                for tin in range(GRP):
                    t = g * GRP + tin
                    rrep = rpool.tile([P, KT, P], bf16, tag="rrep")
                    nc.scalar.activation(
                        out=rrep[:, :, :],
                        in_=row_sb[:, t * KT : (t + 1) * KT]
                        .unsqueeze(-1)
                        .to_broadcast([P, KT, P]),
                        func=mybir.ActivationFunctionType.Copy,
                    )
                    S = spool.tile([P, KT, P], bf16, tag="S")
                    nc.vector.tensor_tensor(
                        out=S[:, :, :],
                        in0=rrep[:, :, :],
                        in1=iota_rep[:, :, :],
                        op=mybir.AluOpType.is_equal,
                    )
                    psum = ppool.tile([P, D], f32, tag="psum")
                    for k in range(KT):
                        if k < NSHARD * cap:
                            s, c = k // cap, k % cap
                            rhs = Gs[s][:, tin * cap + c, 0:D]
                        elif k < NSHARD * (cap + GOC):
                            kk = k - NSHARD * cap
                            s, co = kk // GOC, kk % GOC
                            rhs = Gs[s][:, GRP * cap + co, 0:D]
                        else:
                            kk = k - NSHARD * (cap + GOC)
                            s, co = kk // OC2, kk % OC2
                            rhs = Gov[s][:, co, 0:D]
                        nc.tensor.matmul(
                            out=psum[:, :],
                            lhsT=S[:, k, :],
                            rhs=rhs,
                            start=(k == 0),
                            stop=(k == KT - 1),
                        )
                    psums.append(psum)
                    if tin >= DLY:
                        te = tin - DLY
                        nc.vector.scalar_tensor_tensor(
                            out=og[:, te * D : (te + 1) * D],
                            in0=psums[te][:, 0:D],
                            scalar=rc_sb[:, g * GRP + te : g * GRP + te + 1],
                            in1=slab_sb[:, (g * GRP + te) * D : (g * GRP + te + 1) * D],
                            op0=mybir.AluOpType.mult,
                            op1=mybir.AluOpType.add,
                        )
                for te in range(max(0, GRP - DLY), GRP):
                    nc.vector.scalar_tensor_tensor(
                        out=og[:, te * D : (te + 1) * D],
                        in0=psums[te][:, 0:D],
                        scalar=rc_sb[:, g * GRP + te : g * GRP + te + 1],
                        in1=slab_sb[:, (g * GRP + te) * D : (g * GRP + te + 1) * D],
                        op0=mybir.AluOpType.mult,
                        op1=mybir.AluOpType.add,
                    )
                nc.sync.dma_start(
                    out=ot[:, g * GRP * D : (g + 1) * GRP * D], in_=og[:, :]
                )
    nc.finalize()
    return nc


# ----------------------------------------------------------------- entry


def kernel(features, row, col):
    features = np.asarray(features, dtype=np.float32)
    n_cores = 8
    in_maps, meta = preprocess(features, row, col, n_cores)
    nc = build(meta)

    from concourse.bass_utils import run_bass_kernel_spmd

    res = run_bass_kernel_spmd(nc, in_maps, core_ids=list(range(n_cores)))
    return postprocess(res.results, meta)


if __name__ == "__main__":
    rng = np.random.default_rng(0)
    N, D, E = 7168, 64, 57344
    feats = rng.standard_normal((N, D), dtype=np.float32)
    row = rng.integers(0, N, E, dtype=np.int32)
    col = rng.integers(0, N, E, dtype=np.int32)
    out = kernel(feats, row, col)

    gathered = feats[col]
    summed = np.zeros((N, D), np.float32)
    np.add.at(summed, row, gathered)
    deg = np.clip(np.bincount(row, minlength=N).astype(np.float32), 1.0, None)
    exp = feats + summed / deg[:, None]
    rel = np.linalg.norm(out - exp) / np.linalg.norm(exp)
    print("rel err:", rel, "PASS" if rel < 5e-3 else "FAIL")



# revision 45
# speedup vs baseline: 1.1825x; 1.1825x over previous
"""Bass/Trainium2 kernel for BasicGNNLayer (COO SpMM + mean aggregation + residual).

    out = features + (segment_sum(features[col], row) / clip(deg, 1)) .

Strategy (8 NeuronCores, SPMD, no collectives):
  - Destination-shard nodes: core m owns a 12544-row slab (98 tiles of 128).
  - Host sorts edges by dst row into 128-node tiles; within a tile edges are
    bucketed into 4 OVERLAPPING source windows of 32768 rows (int16 index
    reach). A per-tile balancer assigns window-flexible edges to even out the
    4 buckets, halving cap overflow. Each (tile, window) bucket is capped at
    CAP chunks of 128 edges; the excess spills into per-(7-tile-group, window)
    overflow chunks shared by the group's tiles.
  - The hard bottleneck is Q7 SWDGE descriptor generation (~7-9ns/index,
    data-independent). The 4 windows' gathers go to SWDGE queues 0-3, which
    execute on DIFFERENT Q7 core pairs concurrently (~3.4x).
  - Per (group, window): ONE dma_gather of (GRP*CAP+GOC)*128 indices (256B
    rows from the [N, 128] bf16 table). Per tile: the Scalar engine
    materializes the dst-offset row as a dense [P, KT, 128] tile (so the DVE
    is_equal runs in 2x_1P mode and stays off the Q7's shared SBUF port),
    then chained matmuls S.T @ G accumulate the segment sums in PSUM. The
    epilogue fuses host-precomputed 1/max(deg,1) scaling + residual add in
    one DVE op. Group g's compute hides under group g+1's gather wave;
    gpool bufs=3 keeps the gather waves back-to-back (299ns gaps).
"""

import os
import sys

for _p in ("/opt/trn_rl_repo", "/root/.axon_site/_ro/trn_rl_repo"):
    if os.path.isdir(_p) and _p not in sys.path:
        sys.path.insert(0, _p)

import numpy as np
import ml_dtypes

P = 128  # SBUF partitions
NSHARD = 4  # gather-table shards (int16 index limit)
CAP = 4  # capped chunks per (tile, shard) bucket


def _pick_grp(T):
    for g in range(8, 0, -1):
        if T % g == 0:
            return g
    return 1


def _shard_size(N):
    s = (N + NSHARD - 1) // NSHARD
    assert s <= 32768, "int16 index limit"
    return s


WW = 1 << 15  # shard window width (int16 index limit)


def _window_starts(N):
    # 4 overlapping windows covering [0, N); overlap gives the balancer
    # freedom to even out per-(tile, shard) bucket loads.
    if N > WW:
        base = N - WW
        return np.array([round(i * base / (NSHARD - 1)) for i in range(NSHARD)])
    q = (N + NSHARD - 1) // NSHARD
    return np.array([min(i * q, N - 1) for i in range(NSHARD)])


def _assign_shards(row, col, N):
    """Per-edge shard in 0..3, balancing each dst tile's 4 buckets."""
    starts = _window_starts(N)
    tile = row // P
    s_hi = np.searchsorted(starts, col, side="right") - 1
    flex = (s_hi > 0) & (col < starts[s_hi - 1] + WW)
    ntile = int(tile.max()) + 1
    fixed_cnt = np.zeros((ntile, NSHARD), np.int64)
    np.add.at(fixed_cnt, (tile[~flex], s_hi[~flex]), 1)
    flex_cnt = np.zeros((ntile, NSHARD - 1), np.int64)
    np.add.at(flex_cnt, (tile[flex], s_hi[flex] - 1), 1)

    tvals = np.zeros((ntile, NSHARD - 1), np.int64)
    for tt in range(ntile):
        f = fixed_cnt[tt]
        x = flex_cnt[tt]
        lo, hi = -(-(f.sum() + x.sum()) // NSHARD), int(f.sum() + x.sum())
        best = None
        while lo <= hi:
            M = (lo + hi) // 2
            t = np.zeros(NSHARD - 1, np.int64)
            carry = 0
            ok = True
            for p in range(NSHARD - 1):
                room = M - f[p] - carry
                if room < 0:
                    ok = False
                    break
                t[p] = min(x[p], room)
                carry = x[p] - t[p]
            if ok and f[NSHARD - 1] + carry <= M:
                best = t.copy()
                hi = M - 1
            else:
                lo = M + 1
        assert best is not None
        tvals[tt] = best

    # rank flex edges within (tile, pair) by col; first tvals go to lower shard
    shard = s_hi.copy()
    fi = np.where(flex)[0]
    pair = s_hi[fi] - 1
    key = tile[fi] * (NSHARD - 1) + pair
    order = np.lexsort((col[fi], key))
    ks = key[order]
    grp_start = np.zeros(ntile * (NSHARD - 1), np.int64)
    cnt = np.bincount(ks, minlength=ntile * (NSHARD - 1))
    grp_start[1:] = np.cumsum(cnt)[:-1]
    rank = np.arange(fi.shape[0]) - grp_start[ks]
    down = rank < tvals.reshape(-1)[ks]
    pair_o = pair[order]
    sh_f = np.where(down, pair_o, pair_o + 1)
    shard[fi[order]] = sh_f
    return shard, starts


# ---------------------------------------------------------------- host side


def preprocess(features, row, col, n_cores):
    """Build per-core input maps. Returns (in_maps, meta)."""
    N, D = features.shape
    E = row.shape[0]
    npc = ((N + n_cores - 1) // n_cores + P - 1) // P * P
    T = npc // P
    GRP = _pick_grp(T)
    NG = T // GRP
    SS = _shard_size(N)

    row = np.asarray(row).astype(np.int64)
    col = np.asarray(col).astype(np.int64)

    shard, wst = _assign_shards(row, col, N)
    gts0 = (row // P) * NSHARD + shard  # global (tile, shard) bucket id
    # secondary col sort inside each bucket -> ascending gather addresses
    # (better HBM row-buffer locality for the SDMA drain)
    order = np.argsort(gts0 * (1 << 17) + col, kind="stable")
    rs = row[order]
    cs = col[order]
    sh = shard[order]
    gts = gts0[order]

    n_gts = (n_cores * T) * NSHARD
    cnt = np.bincount(gts, minlength=n_gts)
    Cs = max(1, int((cnt.max() + P - 1) // P))
    cap = min(CAP, Cs)

    # rank of each edge within its bucket
    starts = np.zeros(n_gts, np.int64)
    starts[1:] = np.cumsum(cnt)[:-1]
    pos = np.arange(E) - starts[gts]

    # overflow edges (pos >= cap*128) pool per (core, group, shard)
    ovf = pos >= cap * P
    gtile = rs // P  # global tile id
    core_of = gtile // T
    g_of = (gtile % T) // GRP
    pool = (core_of * NG + g_of) * NSHARD + sh  # global pool id
    n_pools = n_cores * NG * NSHARD
    ovf_pool = pool[ovf]
    po = np.argsort(ovf_pool, kind="stable")
    pcnt = np.bincount(ovf_pool, minlength=n_pools)
    pstart = np.zeros(n_pools, np.int64)
    pstart[1:] = np.cumsum(pcnt)[:-1]
    ovrank_sub = np.empty(ovf_pool.shape[0], np.int64)
    ovrank_sub[po] = np.arange(ovf_pool.shape[0]) - pstart[ovf_pool[po]]
    ovrank = np.zeros(E, np.int64)
    ovrank[np.where(ovf)[0]] = ovrank_sub
    GOC = int((pcnt.max() + P - 1) // P) if ovf.any() else 0
    OC2 = 0
    ov2 = np.zeros(E, bool)
    ovrank2 = np.zeros(E, np.int64)
    if GOC > 1:
        GOC = 1
        ov2[np.where(ovf)[0]] = ovrank_sub >= P
        pool2 = core_of * NSHARD + sh
        p2 = pool2[ov2]
        po2 = np.argsort(p2, kind="stable")
        p2cnt = np.bincount(p2, minlength=n_cores * NSHARD)
        p2start = np.zeros(n_cores * NSHARD, np.int64)
        p2start[1:] = np.cumsum(p2cnt)[:-1]
        r2 = np.empty(p2.shape[0], np.int64)
        r2[po2] = np.arange(p2.shape[0]) - p2start[p2[po2]]
        ovrank2[np.where(ov2)[0]] = r2
        OC2 = int((p2cnt.max() + P - 1) // P) if ov2.any() else 0

    CPT = NSHARD * cap  # capped chunks per tile
    KT = CPT + NSHARD * GOC + NSHARD * OC2  # matmul chunks per tile
    CC = GRP * cap + GOC  # gather chunks per (group, shard) call
    CL = CC * P  # indices per call
    CLW = CL // 16

    tab = np.zeros((N, 2 * D), ml_dtypes.bfloat16)
    tab[:, :D] = features.astype(ml_dtypes.bfloat16)
    deg = np.bincount(row, minlength=N).astype(np.float32)
    recip_full = 1.0 / np.maximum(deg, 1.0)
    iota = np.ascontiguousarray(
        np.broadcast_to(np.arange(P, dtype=np.float32), (P, P))
    ).astype(ml_dtypes.bfloat16)

    bounds = np.searchsorted(gts, np.arange(n_cores + 1) * T * NSHARD)

    in_maps = []
    for m in range(n_cores):
        lo, hi = bounds[m], bounds[m + 1]
        local = rs[lo:hi] - m * npc
        tile = local // P  # tile within core
        shd = sh[lo:hi]
        pp_ = pos[lo:hi]
        ov_ = ovf[lo:hi]
        ovr = ovrank[lo:hi]  # only valid where ov_
        ov2_ = ov2[lo:hi]
        ovr2 = ovrank2[lo:hi]
        g = tile // GRP
        tin = tile % GRP

        # --- capped slots
        cm = ~ov_
        cc = pp_[cm] // P
        cp = pp_[cm] % P
        # rowv column layout per tile t: [s*cap + c | CPT + s*GOC + co]
        rowv = np.full((P, T * KT), -1.0, np.float32)
        rowv[cp, (tile[cm] * KT + shd[cm] * cap + cc)] = (
            local[cm] - tile[cm] * P
        ).astype(np.float32)
        # idx flat position within call (g, s): (tin*cap + cc)*128 + p
        idx_flat = np.zeros(NG * NSHARD * CL, np.int16)
        call = g * NSHARD + shd
        q = (call[cm] * CC + tin[cm] * cap + cc) * P + cp
        idx_flat[q] = (cs[lo:hi][cm] - wst[shd[cm]]).astype(np.int16)

        # --- group overflow slots (first level)
        lvl1 = ov_ & ~ov2_
        if GOC:
            co = ovr[lvl1] // P
            op_ = ovr[lvl1] % P
            rowv[op_, (tile[lvl1] * KT + CPT + shd[lvl1] * GOC + co)] = (
                local[lvl1] - tile[lvl1] * P
            ).astype(np.float32)
            qo = (call[lvl1] * CC + GRP * cap + co) * P + op_
            idx_flat[qo] = (cs[lo:hi][lvl1] - wst[shd[lvl1]]).astype(np.int16)

        ncalls = NG * NSHARD
        w = idx_flat.reshape(ncalls, CLW, 16)
        w = np.ascontiguousarray(np.transpose(w, (2, 0, 1))).reshape(16, ncalls * CLW)
        idx16 = np.ascontiguousarray(np.tile(w, (8, 1)))

        # --- core overflow (second level): 4 mini-calls appended after
        if OC2:
            idx_ov = np.zeros(NSHARD * OC2 * P, np.int16)
            co2 = ovr2[ov2_] // P
            op2 = ovr2[ov2_] % P
            rowv[op2, (tile[ov2_] * KT + CPT + NSHARD * GOC + shd[ov2_] * OC2 + co2)] = (
                local[ov2_] - tile[ov2_] * P
            ).astype(np.float32)
            q2 = (shd[ov2_] * OC2 + co2) * P + op2
            idx_ov[q2] = (cs[lo:hi][ov2_] - wst[shd[ov2_]]).astype(np.int16)
            OW2 = OC2 * P // 16
            w2 = idx_ov.reshape(NSHARD, OW2, 16)
            w2 = np.ascontiguousarray(np.transpose(w2, (2, 0, 1))).reshape(16, NSHARD * OW2)
            idx16 = np.concatenate([idx16, np.tile(w2, (8, 1))], axis=1)

        base = m * npc
        valid = max(0, min(npc, N - base))
        slab = np.zeros((T * P, D), np.float32)
        slab[:valid] = features[base : base + valid]
        feats_loc = np.ascontiguousarray(
            slab.reshape(T, P, D).transpose(1, 0, 2).reshape(P, T * D)
        )
        slabr = np.ones(T * P, np.float32)
        slabr[:valid] = recip_full[base : base + valid]
        recip_loc = np.ascontiguousarray(slabr.reshape(T, P).T)

        in_maps.append(
            {
                "tab": tab,
                "feats_loc": feats_loc,
                "idx16": idx16,
                "rowv": rowv.astype(ml_dtypes.bfloat16),
                "iota": iota,
                "recip": recip_loc,
            }
        )

    meta = dict(N=N, D=D, E=E, npc=npc, T=T, cap=cap, GOC=GOC, OC2=OC2, KT=KT,
                CC=CC, NG=NG, SS=SS, GRP=GRP, n_cores=n_cores,
                starts=[int(s) for s in wst])
    return in_maps, meta


def postprocess(results, meta):
    N, D, npc, T = meta["N"], meta["D"], meta["npc"], meta["T"]
    outs = []
    for m, res in enumerate(results):
        o = res["out"].reshape(P, T, D).transpose(1, 0, 2).reshape(T * P, D)
        valid = max(0, min(npc, N - m * npc))
        outs.append(o[:valid])
    return np.concatenate(outs, axis=0)


# -------------------------------------------------------------- device side


def build(meta):
    import concourse.bass as bass  # noqa: F401
    import concourse.bacc as bacc
    import concourse.mybir as mybir
    from concourse.tile import TileContext

    N, D, T = meta["N"], meta["D"], meta["T"]
    cap, GOC, KT, CC = meta["cap"], meta["GOC"], meta["KT"], meta["CC"]
    OC2 = meta["OC2"]
    NG, SS, GRP = meta["NG"], meta["SS"], meta["GRP"]
    starts = meta["starts"]
    W = 2 * D  # table row width (128)
    bf16 = mybir.dt.bfloat16
    f32 = mybir.dt.float32

    nc = bacc.Bacc(num_swdge_queues=4)
    tab = nc.dram_tensor("tab", [N, W], bf16, kind="ExternalInput")
    fl = nc.dram_tensor("feats_loc", [P, T * D], f32, kind="ExternalInput")
    NIX = NG * NSHARD * CC * P // 16 + NSHARD * OC2 * P // 16
    ix = nc.dram_tensor("idx16", [P, NIX], mybir.dt.int16, kind="ExternalInput")
    rv = nc.dram_tensor("rowv", [P, T * KT], bf16, kind="ExternalInput")
    io = nc.dram_tensor("iota", [P, P], bf16, kind="ExternalInput")
    rc = nc.dram_tensor("recip", [P, T], f32, kind="ExternalInput")
    ot = nc.dram_tensor("out", [P, T * D], f32, kind="ExternalOutput")

    CLW = CC * P // 16

    with TileContext(nc) as tc:
        with (
            tc.tile_pool(name="const", bufs=1) as cpool,
            tc.tile_pool(name="gat", bufs=3) as gpool,
            tc.tile_pool(name="rep", bufs=4) as rpool,
            tc.tile_pool(name="sel", bufs=6) as spool,
            tc.tile_pool(name="eplg", bufs=4) as epool,
            tc.tile_pool(name="acc", bufs=8, space="PSUM") as ppool,
        ):
            iota_sb = cpool.tile([P, P], bf16, tag="iota")
            nc.sync.dma_start(out=iota_sb[:, :], in_=io[:, :])
            # all group index tables resident up-front: no per-group idx DMA
            # on the critical chain; group 0's slice loads separately first so
            # the first gather wave is not gated on the full 3 MB transfer
            ixall = cpool.tile([P, NG * NSHARD * CLW], mybir.dt.int16, tag="ixall")
            nc.sync.dma_start(
                out=ixall[:, : NSHARD * CLW], in_=ix[:, : NSHARD * CLW]
            )
            if NG > 1:
                nc.sync.dma_start(
                    out=ixall[:, NSHARD * CLW : NG * NSHARD * CLW],
                    in_=ix[:, NSHARD * CLW : NG * NSHARD * CLW],
                )
            # dense [P, KT, P] iota replica: gives the is_equal two step-1
            # bf16 operands so the DVE can run in 2x_1P mode.
            iota_rep = cpool.tile([P, KT, P], bf16, tag="iota_rep")
            nc.scalar.activation(
                out=iota_rep[:, :, :],
                in_=iota_sb[:, :].unsqueeze(1).to_broadcast([P, KT, P]),
                func=mybir.ActivationFunctionType.Copy,
            )
            # bulk constants go through the Activation HWDGE ring so the Sync
            # ring can issue the first group's index loads immediately
            row_sb = cpool.tile([P, T * KT], bf16, tag="rowsb")
            nc.scalar.dma_start(out=row_sb[:, :], in_=rv[:, :])
            slab_sb = cpool.tile([P, T * D], f32, tag="slab")
            nc.scalar.dma_start(out=slab_sb[:, :], in_=fl[:, :])
            rc_sb = cpool.tile([P, T], f32, tag="recip")
            nc.scalar.dma_start(out=rc_sb[:, :], in_=rc[:, :])
            Gov = []
            if OC2:
                OW2 = OC2 * P // 16
                ixo = cpool.tile([P, NSHARD * OW2], mybir.dt.int16, tag="ixo")
                nc.sync.dma_start(
                    out=ixo[:, :],
                    in_=ix[:, NG * NSHARD * CC * P // 16 :],
                )
                for s in range(NSHARD):
                    Gv = cpool.tile([P, OC2, W], bf16, tag=f"Gov{s}")
                    nc.gpsimd.dma_gather(
                        out_ap=Gv[:, :, :],
                        in_ap=tab[starts[s] : min(N, starts[s] + WW), :],
                        idxs_ap=ixo[:, s * OW2 : (s + 1) * OW2],
                        num_idxs=OC2 * P,
                        num_idxs_reg=OC2 * P,
                        elem_size=W,
                        single_packet=False,
                        queue_num=s % 4,
                    )
                    Gov.append(Gv)

            for g in range(NG):
                Gs = []
                for s in range(NSHARD):
                    Gt = gpool.tile([P, CC, W], bf16, tag=f"G{s}")
                    call = g * NSHARD + s
                    nc.gpsimd.dma_gather(
                        out_ap=Gt[:, :, :],
                        in_ap=tab[starts[s] : min(N, starts[s] + WW), :],
                        idxs_ap=ixall[:, call * CLW : (call + 1) * CLW],
                        num_idxs=CC * P,
                        num_idxs_reg=CC * P,
                        elem_size=W,
                        single_packet=False,
                        queue_num=s % 4,
                    )
                    Gs.append(Gt)
                og = epool.tile([P, GRP * D], f32, tag="og")
                # Epilogues trail the S-build/matmul of their tile by 2 in the
                # DVE stream: eplg(t) then never blocks is_eq(t+1) on tile t's
                # matmul chain (the serial DVE->PE->DVE cycle that otherwise
                # paces the group at ~3us/tile), while keeping the DVE work
                # spread out instead of bursted (bursts slow the Q7 desc-gen).
                DLY = 2
                psums = []
                for tin in range(GRP):
                    t = g * GRP + tin
                    rrep = rpool.tile([P, KT, P], bf16, tag="rrep")
                    nc.scalar.activation(
                        out=rrep[:, :, :],
                        in_=row_sb[:, t * KT : (t + 1) * KT]
                        .unsqueeze(-1)
                        .to_broadcast([P, KT, P]),
                        func=mybir.ActivationFunctionType.Copy,
                    )
                    S = spool.tile([P, KT, P], bf16, tag="S")
                    nc.vector.tensor_tensor(
                        out=S[:, :, :],
                        in0=rrep[:, :, :],
                        in1=iota_rep[:, :, :],
                        op=mybir.AluOpType.is_equal,
                    )
                    psum = ppool.tile([P, D], f32, tag="psum")
                    for k in range(KT):
                        if k < NSHARD * cap:
                            s, c = k // cap, k % cap
                            rhs = Gs[s][:, tin * cap + c, 0:D]
                        elif k < NSHARD * (cap + GOC):
                            kk = k - NSHARD * cap
                            s, co = kk // GOC, kk % GOC
                            rhs = Gs[s][:, GRP * cap + co, 0:D]
                        else:
                            kk = k - NSHARD * (cap + GOC)
                            s, co = kk // OC2, kk % OC2
                            rhs = Gov[s][:, co, 0:D]
                        nc.tensor.matmul(
                            out=psum[:, :],
                            lhsT=S[:, k, :],
                            rhs=rhs,
                            start=(k == 0),
                            stop=(k == KT - 1),
                        )
                    psums.append(psum)
                    if tin >= DLY:
                        te = tin - DLY
                        nc.vector.scalar_tensor_tensor(
                            out=og[:, te * D : (te + 1) * D],
                            in0=psums[te][:, 0:D],
                            scalar=rc_sb[:, g * GRP + te : g * GRP + te + 1],
                            in1=slab_sb[:, (g * GRP + te) * D : (g * GRP + te + 1) * D],
                            op0=mybir.AluOpType.mult,
                            op1=mybir.AluOpType.add,
                        )
                for te in range(max(0, GRP - DLY), GRP):
                    nc.vector.scalar_tensor_tensor(
                        out=og[:, te * D : (te + 1) * D],
                        in0=psums[te][:, 0:D],
                        scalar=rc_sb[:, g * GRP + te : g * GRP + te + 1],
                        in1=slab_sb[:, (g * GRP + te) * D : (g * GRP + te + 1) * D],
                        op0=mybir.AluOpType.mult,
                        op1=mybir.AluOpType.add,
                    )
                nc.sync.dma_start(
                    out=ot[:, g * GRP * D : (g + 1) * GRP * D], in_=og[:, :]
                )
    nc.finalize()
    return nc


# ----------------------------------------------------------------- entry


def kernel(features, row, col):
    features = np.asarray(features, dtype=np.float32)
    n_cores = 8
    in_maps, meta = preprocess(features, row, col, n_cores)
    nc = build(meta)

    from concourse.bass_utils import run_bass_kernel_spmd

    res = run_bass_kernel_spmd(nc, in_maps, core_ids=list(range(n_cores)))
    return postprocess(res.results, meta)


if __name__ == "__main__":
    rng = np.random.default_rng(0)
    N, D, E = 7168, 64, 57344
    feats = rng.standard_normal((N, D), dtype=np.float32)
    row = rng.integers(0, N, E, dtype=np.int32)
    col = rng.integers(0, N, E, dtype=np.int32)
    out = kernel(feats, row, col)

    gathered = feats[col]
    summed = np.zeros((N, D), np.float32)
    np.add.at(summed, row, gathered)
    deg = np.clip(np.bincount(row, minlength=N).astype(np.float32), 1.0, None)
    exp = feats + summed / deg[:, None]
    rel = np.linalg.norm(out - exp) / np.linalg.norm(exp)
    print("rel err:", rel, "PASS" if rel < 5e-3 else "FAIL")

